# revision 7
# baseline (speedup 1.0000x reference)
"""Distributed 3-layer GAT on 8 TRN2 NeuronCores (Bass/Tile).

Sharding: core c owns dst nodes [c*NS, (c+1)*NS). Edges partitioned by dst
shard, sorted by (src-chunk, dst-chunk, dst) with sc-major tile numbering.
Weights replicated. Per layer: sharded dense (h @ [W|ws|wd]) -> AllGather
768B-row gather table -> edge phase: bulk dma_gather of src rows (4 SWDGE
queues), one-hot eq matmul accumulates weighted messages + softmax denom in
PSUM, eqT (PE transpose) matmul expands s_dst to edges. Normalization
per node after accumulation (division commutes with segment sum; amax
subtraction skipped: |alpha| <~ 6 so exp() is safe).
"""
import os, sys, types
sys.path.insert(0, "/opt/trn_rl_repo")
import numpy as np

import antenv
if "antenv.axon_hooks" not in sys.modules:
    _hooks_mod = types.ModuleType("antenv.axon_hooks")
    _HOOK = [None]
    _hooks_mod.set_axon_ntff_profile_hook = lambda h: _HOOK.__setitem__(0, h)
    _hooks_mod.get_axon_ntff_profile_hook = lambda: _HOOK[0]
    sys.modules["antenv.axon_hooks"] = _hooks_mod
    antenv.axon_hooks = _hooks_mod
    try:
        import trn_agent_boot.trn_boot as _tb
        _h = _tb._ntff_profile_via_ctypes("/opt/axon/libaxon_pjrt.so")
        if _h is not None:
            _hooks_mod.set_axon_ntff_profile_hook(_h)
    except Exception:
        pass

from concourse import bass, bacc, mybir, tile
from concourse import bass_utils
from concourse.bass_utils import run_bass_kernel_spmd
from concourse._compat import get_trn_type
bass_utils.upload_artifacts = lambda tmpdir: "local://noop"

F32 = mybir.dt.float32
I16 = mybir.dt.int16
P = 128
SLOPE = 0.2

# problem constants (hardcoded per contract; debug scripts may override)
N = 100000
IN, HID, HEADS, OUT = 128, 32, 4, 32
NCORES = 8
NS = N // NCORES              # nodes per shard
NCH = (NS + P - 1) // P       # dst chunks per core
S_CH = 3                      # dst chunks per super-chunk
NSC = 4                       # src table chunks (int16 limit)
CH = N // NSC                 # rows per src chunk
ROWF = 192                    # gather row f32 elems (768B, %256==0)
FEAT = 128
NCOL = 136                    # xw(128) + s_src(4) + s_dst(4)


def _build_wext(W, a_s, a_d):
    Fin = W.shape[0]
    H, C = a_s.shape
    Wr = W.reshape(Fin, H, C)
    ws = np.einsum("fhc,hc->fh", Wr, a_s)
    wd = np.einsum("fhc,hc->fh", Wr, a_d)
    return np.ascontiguousarray(np.concatenate([W, ws, wd], axis=1), np.float32)


def _prep_graph(edge_index):
    loops = np.arange(N, dtype=np.int64)
    src = np.concatenate([edge_index[0].astype(np.int64), loops])
    dst = np.concatenate([edge_index[1].astype(np.int64), loops])

    core = dst // NS
    per_core = []
    counts = np.zeros((NCORES, NCH, NSC), dtype=np.int64)
    for c in range(NCORES):
        m = core == c
        s_c, d_c = src[m], dst[m]
        dl = d_c - c * NS
        ch = dl // P
        sc = s_c // CH
        order = np.lexsort((d_c, ch, sc))  # sc-major, then chunk, then dst
        s_c, dl, ch, sc = s_c[order], dl[order], ch[order], sc[order]
        np.add.at(counts[c], (ch, sc), 1)
        per_core.append((s_c, dl, ch, sc))

    T = np.ceil(counts.max(axis=0) / P).astype(np.int64)   # [NCH, NSC]
    # sc-major tile numbering: tiles for (sc, ch) laid out sc outer, ch inner,
    # so one super-chunk x src-chunk span is contiguous.
    tile_off = np.zeros((NCH, NSC), dtype=np.int64)
    acc = 0
    for sc in range(NSC):
        for ch in range(NCH):
            tile_off[ch, sc] = acc
            acc += T[ch, sc]
    ntiles = acc

    gidx = np.zeros((NCORES, ntiles * P), dtype=np.int16)
    dstloc = np.full((NCORES, ntiles, P), -1.0, dtype=np.float32)
    for c in range(NCORES):
        s_c, dl, ch, sc = per_core[c]
        # edges are sorted (sc, ch, dst); walk runs in the same order
        pos = 0
        for scv in range(NSC):
            for chv in range(NCH):
                n = int(counts[c, chv, scv])
                if n == 0:
                    continue
                t0 = int(tile_off[chv, scv])
                gidx[c, t0 * P: t0 * P + n] = (s_c[pos:pos + n] - scv * CH).astype(np.int16)
                dstloc[c].reshape(-1)[t0 * P: t0 * P + n] = (dl[pos:pos + n] - chv * P).astype(np.float32)
                pos += n
        assert pos == len(s_c)
    return T, tile_off, int(ntiles), gidx, dstloc


def _wrap_idx(flat16):
    n = flat16.shape[0]
    w = flat16.reshape(n // 16, 16).T
    return np.ascontiguousarray(np.tile(w, (8, 1)), np.int16)


def _build_program(T, tile_off, ntiles):
    nc = bacc.Bacc(get_trn_type() or "TRN2", target_bir_lowering=False,
                   debug=False, enable_asserts=False, num_devices=NCORES,
                   num_swdge_queues=4)
    x_sh = nc.dram_tensor("x_shard", [NS, FEAT], F32, kind="ExternalInput").ap()
    gidx_t = nc.dram_tensor("gidx", [P, ntiles * 8], I16, kind="ExternalInput").ap()
    dstloc_t = nc.dram_tensor("dstloc", [P, ntiles], F32, kind="ExternalInput").ap()
    wext_t = [nc.dram_tensor(f"wext{l}", [FEAT, NCOL], F32, kind="ExternalInput").ap() for l in range(3)]
    btile_t = [nc.dram_tensor(f"btile{l}", [P, FEAT if l < 2 else OUT], F32, kind="ExternalInput").ap() for l in range(3)]
    iota_t = nc.dram_tensor("iota", [P, P], F32, kind="ExternalInput").ap()
    ident_t = nc.dram_tensor("ident", [P, P], F32, kind="ExternalInput").ap()
    out_t = nc.dram_tensor("out", [NS, OUT], F32, kind="ExternalOutput").ap()
    DBG = bool(int(os.environ.get("GAT_DEBUG", "0")))
    if DBG:
        dbg_xw = nc.dram_tensor("dbg_xw", [NS, NCOL], F32, kind="ExternalOutput").ap()
        dbg_h = nc.dram_tensor("dbg_h", [NS, FEAT], F32, kind="ExternalOutput").ap()
        dbg_den = nc.dram_tensor("dbg_den", [NS, 4], F32, kind="ExternalOutput").ap()
        dbg_ex = nc.dram_tensor("dbg_ex", [ntiles, P, 4], F32, kind="ExternalOutput").ap()
        dbg_sd = nc.dram_tensor("dbg_sd", [ntiles, P, 4], F32, kind="ExternalOutput").ap()

    groups = [list(range(NCORES))]
    NSUP = (NCH + S_CH - 1) // S_CH

    with tile.TileContext(nc) as tc:
        with (
            tc.tile_pool(name="const", bufs=1) as constp,
            tc.tile_pool(name="sched", bufs=1) as schedp,
            tc.tile_pool(name="gpool", bufs=3) as gpool,
            tc.tile_pool(name="eqp", bufs=2) as eqp,
            tc.tile_pool(name="eqtp", bufs=4) as eqtp,
            tc.tile_pool(name="work", bufs=4) as work,
            tc.tile_pool(name="nodep", bufs=3) as nodep,
            tc.tile_pool(name="accp", bufs=4, space="PSUM") as accp,
            tc.tile_pool(name="tpp", bufs=2, space="PSUM") as tpp,
            tc.tile_pool(name="sdp", bufs=1, space="PSUM") as sdp,
            tc.tile_pool(name="dnp", bufs=1, space="PSUM") as dnp,
            tc.tile_pool(name="dram", bufs=2, space="DRAM") as dramp,
        ):
            iota = constp.tile([P, P], F32, tag="iota")
            nc.sync.dma_start(out=iota[:], in_=iota_t[:])
            ident = constp.tile([P, P], F32, tag="ident")
            nc.sync.dma_start(out=ident[:], in_=ident_t[:])
            wext, btile = [], []
            for l in range(3):
                w = constp.tile([FEAT, NCOL], F32, tag=f"wext{l}")
                nc.sync.dma_start(out=w[:], in_=wext_t[l][:])
                wext.append(w)
                b = constp.tile([P, FEAT if l < 2 else OUT], F32, tag=f"bt{l}")
                nc.sync.dma_start(out=b[:], in_=btile_t[l][:])
                btile.append(b)
            gidx_sb = schedp.tile([P, ntiles * 8], I16, tag="gidx")
            nc.sync.dma_start(out=gidx_sb[:], in_=gidx_t[:])
            dstloc_sb = schedp.tile([P, ntiles], F32, tag="dstloc")
            nc.sync.dma_start(out=dstloc_sb[:], in_=dstloc_t[:])

            xwss_sh = [dramp.tile([NS, ROWF], F32, tag="xwsh", name=f"xwsh{i}") for i in range(3)]
            xwss_full = [dramp.tile([N, ROWF], F32, tag="xwfull", name=f"xwfull{i}", addr_space="Shared") for i in range(3)]
            sd_sh = [dramp.tile([NS, 4], F32, tag="sdsh", name=f"sdsh{i}") for i in range(3)]

            def dense_tile(h_sb, lnext, base, nn):
                hT_ps = tpp.tile([P, P], F32, tag="tp")
                nc.tensor.transpose(out=hT_ps[:], in_=h_sb[:], identity=ident[:])
                hT_sb = work.tile([P, P], F32, tag="hT")
                nc.scalar.copy(out=hT_sb[:], in_=hT_ps[:])
                d_ps = dnp.tile([P, NCOL], F32, tag="dn")
                nc.tensor.matmul(out=d_ps[:], lhsT=hT_sb[:], rhs=wext[lnext][:],
                                 start=True, stop=True)
                xo = work.tile([P, NCOL], F32, tag="xo")
                nc.scalar.copy(out=xo[:], in_=d_ps[:])
                nc.sync.dma_start(out=xwss_sh[lnext][base:base + nn, 0:NCOL],
                                  in_=xo[:nn, :])
                nc.sync.dma_start(out=sd_sh[lnext][base:base + nn, :],
                                  in_=xo[:nn, 132:136])
                if DBG and lnext == 0:
                    nc.sync.dma_start(out=dbg_xw[base:base + nn, :], in_=xo[:nn, :])

            for t in range(NCH):
                base = t * P
                nn = min(P, NS - base)
                xt = work.tile([P, FEAT], F32, tag="xt")
                nc.sync.dma_start(out=xt[:nn, :], in_=x_sh[base:base + nn, :])
                dense_tile(xt, 0, base, nn)

            for layer in range(3):
                nc.gpsimd.collective_compute(
                    "AllGather", mybir.AluOpType.bypass, replica_groups=groups,
                    ins=[xwss_sh[layer].opt()], outs=[xwss_full[layer].opt()])

                for sup in range(NSUP):
                    ch0 = sup * S_CH
                    chn = min(S_CH, NCH - ch0)

                    sdch = []
                    for ci in range(chn):
                        base = (ch0 + ci) * P
                        nn = min(P, NS - base)
                        s = work.tile([P, 4], F32, tag="sd")
                        nc.sync.dma_start(out=s[:nn, :], in_=sd_sh[layer][base:base + nn, :])
                        sdch.append(s)

                    # per src-chunk: contiguous tile span + one dma_gather
                    spans = []
                    for sc in range(NSC):
                        t0 = int(tile_off[ch0, sc])
                        ncall = int(sum(T[ch0 + ci, sc] for ci in range(chn)))
                        spans.append((t0, ncall))

                    acc_ps = [accp.tile([P, NCOL], F32, tag="acc", name=f"acc_l{layer}s{sup}c{ci}") for ci in range(chn)]
                    mm_count = [0] * chn
                    mm_total = [int(T[ch0 + ci, :].sum()) for ci in range(chn)]

                    for sc in range(NSC):
                        t0, ncall = spans[sc]
                        if ncall == 0:
                            continue
                        gout = gpool.tile([P, ncall, ROWF], F32, tag="g")
                        nc.gpsimd.dma_gather(
                            out_ap=gout[:],
                            in_ap=xwss_full[layer][sc * CH:(sc + 1) * CH, :],
                            idxs_ap=gidx_sb[:, t0 * 8:(t0 + ncall) * 8],
                            num_idxs=ncall * P, num_idxs_reg=ncall * P,
                            elem_size=ROWF, single_packet=False, queue_num=sc % 4)

                        eqg = eqp.tile([P, ncall, P], F32, tag="eq")
                        sd_ps = sdp.tile([P, ncall, 4], F32, tag="sdps")
                        slot = 0
                        tlist = []
                        for ci in range(chn):
                            for _ in range(int(T[ch0 + ci, sc])):
                                tg = t0 + slot
                                nc.vector.tensor_scalar(
                                    out=eqg[:, slot, :], in0=iota[:],
                                    scalar1=dstloc_sb[:, tg:tg + 1], scalar2=None,
                                    op0=mybir.AluOpType.is_equal)
                                eqT_ps = tpp.tile([P, P], F32, tag="tp")
                                nc.tensor.transpose(out=eqT_ps[:], in_=eqg[:, slot, :],
                                                    identity=ident[:])
                                eqT = eqtp.tile([P, P], F32, tag="eqT")
                                nc.scalar.copy(out=eqT[:], in_=eqT_ps[:])
                                nc.tensor.matmul(out=sd_ps[:, slot, :], lhsT=eqT[:],
                                                 rhs=sdch[ci][:], start=True, stop=True)
                                tlist.append(ci)
                                slot += 1

                        al = work.tile([P, ncall, 4], F32, tag="al")
                        nc.vector.tensor_tensor(out=al[:], in0=gout[:, :, 128:132],
                                                in1=sd_ps[:], op=mybir.AluOpType.add)
                        al2 = work.tile([P, ncall, 4], F32, tag="al2")
                        nc.vector.tensor_scalar(out=al2[:], in0=al[:], scalar1=SLOPE,
                                                scalar2=None, op0=mybir.AluOpType.mult)
                        nc.vector.tensor_tensor(out=al[:], in0=al[:], in1=al2[:],
                                                op=mybir.AluOpType.max)
                        nc.scalar.activation(out=gout[:, :, 132:136], in_=al[:],
                                             func=mybir.ActivationFunctionType.Exp)
                        nc.vector.tensor_tensor(
                            out=gout[:, :, 0:128].rearrange("p t (h c) -> p t h c", h=4),
                            in0=gout[:, :, 0:128].rearrange("p t (h c) -> p t h c", h=4),
                            in1=gout[:, :, 132:136].unsqueeze(3).broadcast_to([P, ncall, 4, 32]),
                            op=mybir.AluOpType.mult)

                        if DBG and layer == 0:
                            sdc = work.tile([P, ncall, 4], F32, tag="sdc")
                            nc.vector.tensor_copy(out=sdc[:], in_=sd_ps[:])
                            nc.sync.dma_start(out=dbg_sd[t0:t0 + ncall].rearrange("t p f -> p t f"), in_=sdc[:])
                            nc.sync.dma_start(out=dbg_ex[t0:t0 + ncall].rearrange("t p f -> p t f"), in_=gout[:, :, 132:136])
                        for slot, ci in enumerate(tlist):
                            mm_count[ci] += 1
                            nc.tensor.matmul(
                                out=acc_ps[ci][:], lhsT=eqg[:, slot, :],
                                rhs=gout[:, slot, 0:NCOL],
                                start=(mm_count[ci] == 1),
                                stop=(mm_count[ci] == mm_total[ci]))

                    for ci in range(chn):
                        base = (ch0 + ci) * P
                        nn = min(P, NS - base)
                        r = work.tile([P, 4], F32, tag="r")
                        if DBG and layer == 0:
                            dnt = work.tile([P, 4], F32, tag="dnt")
                            nc.vector.tensor_copy(out=dnt[:], in_=acc_ps[ci][:, 132:136])
                            nc.sync.dma_start(out=dbg_den[base:base + nn, :], in_=dnt[:nn, :])
                        nc.vector.reciprocal(out=r[:], in_=acc_ps[ci][:, 132:136])
                        h = nodep.tile([P, FEAT], F32, tag="h")
                        nc.vector.tensor_tensor(
                            out=h[:].rearrange("p (h c) -> p h c", h=4),
                            in0=acc_ps[ci][:, 0:128].rearrange("p (h c) -> p h c", h=4),
                            in1=r[:].unsqueeze(2).broadcast_to([P, 4, 32]),
                            op=mybir.AluOpType.mult)
                        if layer < 2:
                            nc.vector.tensor_tensor(out=h[:], in0=h[:], in1=btile[layer][:],
                                                    op=mybir.AluOpType.add)
                            mn = nodep.tile([P, FEAT], F32, tag="mn")
                            nc.vector.tensor_scalar(out=mn[:], in0=h[:], scalar1=0.0,
                                                    scalar2=None, op0=mybir.AluOpType.min)
                            nc.scalar.activation(out=mn[:], in_=mn[:],
                                                 func=mybir.ActivationFunctionType.Exp)
                            nc.vector.tensor_scalar(out=h[:], in0=h[:], scalar1=0.0,
                                                    scalar2=None, op0=mybir.AluOpType.max)
                            nc.vector.tensor_tensor(out=h[:], in0=h[:], in1=mn[:],
                                                    op=mybir.AluOpType.add)
                            nc.vector.tensor_scalar(out=h[:], in0=h[:], scalar1=-1.0,
                                                    scalar2=None, op0=mybir.AluOpType.add)
                            if DBG and layer == 0:
                                nc.sync.dma_start(out=dbg_h[base:base + nn, :], in_=h[:nn, :])
                            dense_tile(h, layer + 1, base, nn)
                        else:
                            o = nodep.tile([P, OUT], F32, tag="o")
                            hv = h[:].rearrange("p (h c) -> p h c", h=4)
                            nc.vector.tensor_tensor(out=o[:], in0=hv[:, 0, :], in1=hv[:, 1, :],
                                                    op=mybir.AluOpType.add)
                            nc.vector.tensor_tensor(out=o[:], in0=o[:], in1=hv[:, 2, :],
                                                    op=mybir.AluOpType.add)
                            nc.vector.tensor_tensor(out=o[:], in0=o[:], in1=hv[:, 3, :],
                                                    op=mybir.AluOpType.add)
                            nc.vector.tensor_scalar(out=o[:], in0=o[:], scalar1=0.25,
                                                    scalar2=None, op0=mybir.AluOpType.mult)
                            nc.vector.tensor_tensor(out=o[:], in0=o[:], in1=btile[2][:],
                                                    op=mybir.AluOpType.add)
                            nc.sync.dma_start(out=out_t[base:base + nn, :], in_=o[:nn, :])
    nc.compile()
    return nc


def kernel(x, edge_index, W1, as1, ad1, b1, W2, as2, ad2, b2, W3, as3, ad3, b3):
    x = np.asarray(x, np.float32)
    edge_index = np.asarray(edge_index)
    T, tile_off, ntiles, gidx, dstloc = _prep_graph(edge_index)
    nc = _build_program(T, tile_off, ntiles)

    wext = [_build_wext(np.asarray(W1, np.float32), np.asarray(as1, np.float32), np.asarray(ad1, np.float32)),
            _build_wext(np.asarray(W2, np.float32), np.asarray(as2, np.float32), np.asarray(ad2, np.float32)),
            _build_wext(np.asarray(W3, np.float32), np.asarray(as3, np.float32), np.asarray(ad3, np.float32))]
    bt = [np.ascontiguousarray(np.tile(np.asarray(b, np.float32)[None, :], (P, 1)))
          for b in (b1, b2, b3)]
    iota_np = np.ascontiguousarray(np.tile(np.arange(P, dtype=np.float32)[None, :], (P, 1)))
    ident_np = np.eye(P, dtype=np.float32)

    in_maps = []
    for c in range(NCORES):
        m = {
            "x_shard": np.ascontiguousarray(x[c * NS:(c + 1) * NS]),
            "gidx": _wrap_idx(gidx[c]),
            "dstloc": np.ascontiguousarray(dstloc[c].T),
            "iota": iota_np, "ident": ident_np,
        }
        for l in range(3):
            m[f"wext{l}"] = wext[l]
            m[f"btile{l}"] = bt[l]
        in_maps.append(m)

    trace = bool(int(os.environ.get("GAT_TRACE", "0")))
    res = run_bass_kernel_spmd(nc, in_maps, list(range(NCORES)), trace=trace)
    kernel.last_exec_time_ns = res.exec_time_ns
    out = np.concatenate([res.results[c]["out"] for c in range(NCORES)], axis=0)
    return out


kernel.last_exec_time_ns = None


# revision 8
# speedup vs baseline: 1.0116x; 1.0116x over previous
"""Distributed 3-layer GAT on 8 TRN2 NeuronCores (Bass/Tile).

Sharding: core c owns dst nodes [c*NS, (c+1)*NS). Edges partitioned by dst
shard, sorted by (src-chunk, dst-chunk, dst) with sc-major tile numbering.
Weights replicated. Per layer: sharded dense (h @ [W|ws|wd]) -> AllGather
768B-row gather table -> edge phase: bulk dma_gather of src rows (4 SWDGE
queues), one-hot eq matmul accumulates weighted messages + softmax denom in
PSUM, eqT (PE transpose) matmul expands s_dst to edges. Normalization
per node after accumulation (division commutes with segment sum; amax
subtraction skipped: |alpha| <~ 6 so exp() is safe).
"""
import os, sys, types
sys.path.insert(0, "/opt/trn_rl_repo")
import numpy as np

import antenv
if "antenv.axon_hooks" not in sys.modules:
    _hooks_mod = types.ModuleType("antenv.axon_hooks")
    _HOOK = [None]
    _hooks_mod.set_axon_ntff_profile_hook = lambda h: _HOOK.__setitem__(0, h)
    _hooks_mod.get_axon_ntff_profile_hook = lambda: _HOOK[0]
    sys.modules["antenv.axon_hooks"] = _hooks_mod
    antenv.axon_hooks = _hooks_mod
    try:
        import trn_agent_boot.trn_boot as _tb
        _h = _tb._ntff_profile_via_ctypes("/opt/axon/libaxon_pjrt.so")
        if _h is not None:
            _hooks_mod.set_axon_ntff_profile_hook(_h)
    except Exception:
        pass

from concourse import bass, bacc, mybir, tile
from concourse import bass_utils
from concourse.bass_utils import run_bass_kernel_spmd
from concourse._compat import get_trn_type
bass_utils.upload_artifacts = lambda tmpdir: "local://noop"

F32 = mybir.dt.float32
I16 = mybir.dt.int16
P = 128
SLOPE = 0.2

# problem constants (hardcoded per contract; debug scripts may override)
N = 100000
IN, HID, HEADS, OUT = 128, 32, 4, 32
NCORES = 8
NS = N // NCORES              # nodes per shard
NCH = (NS + P - 1) // P       # dst chunks per core
S_CH = 3                      # dst chunks per super-chunk
NSC = 4                       # src table chunks (int16 limit)
CH = N // NSC                 # rows per src chunk
ROWF = 192                    # gather row f32 elems (768B, %256==0)
FEAT = 128
NCOL = 136                    # xw(128) + s_src(4) + s_dst(4)


def _build_wext(W, a_s, a_d):
    Fin = W.shape[0]
    H, C = a_s.shape
    Wr = W.reshape(Fin, H, C)
    ws = np.einsum("fhc,hc->fh", Wr, a_s)
    wd = np.einsum("fhc,hc->fh", Wr, a_d)
    return np.ascontiguousarray(np.concatenate([W, ws, wd], axis=1), np.float32)


def _prep_graph(edge_index):
    loops = np.arange(N, dtype=np.int64)
    src = np.concatenate([edge_index[0].astype(np.int64), loops])
    dst = np.concatenate([edge_index[1].astype(np.int64), loops])

    core = dst // NS
    per_core = []
    counts = np.zeros((NCORES, NCH, NSC), dtype=np.int64)
    for c in range(NCORES):
        m = core == c
        s_c, d_c = src[m], dst[m]
        dl = d_c - c * NS
        ch = dl // P
        sc = s_c // CH
        order = np.lexsort((d_c, ch, sc))  # sc-major, then chunk, then dst
        s_c, dl, ch, sc = s_c[order], dl[order], ch[order], sc[order]
        np.add.at(counts[c], (ch, sc), 1)
        per_core.append((s_c, dl, ch, sc))

    T = np.ceil(counts.max(axis=0) / P).astype(np.int64)   # [NCH, NSC]
    # sc-major tile numbering: tiles for (sc, ch) laid out sc outer, ch inner,
    # so one super-chunk x src-chunk span is contiguous.
    tile_off = np.zeros((NCH, NSC), dtype=np.int64)
    acc = 0
    for sc in range(NSC):
        for ch in range(NCH):
            tile_off[ch, sc] = acc
            acc += T[ch, sc]
    ntiles = acc

    gidx = np.zeros((NCORES, ntiles * P), dtype=np.int16)
    dstloc = np.full((NCORES, ntiles, P), -1.0, dtype=np.float32)
    for c in range(NCORES):
        s_c, dl, ch, sc = per_core[c]
        # edges are sorted (sc, ch, dst); walk runs in the same order
        pos = 0
        for scv in range(NSC):
            for chv in range(NCH):
                n = int(counts[c, chv, scv])
                if n == 0:
                    continue
                t0 = int(tile_off[chv, scv])
                gidx[c, t0 * P: t0 * P + n] = (s_c[pos:pos + n] - scv * CH).astype(np.int16)
                dstloc[c].reshape(-1)[t0 * P: t0 * P + n] = (dl[pos:pos + n] - chv * P).astype(np.float32)
                pos += n
        assert pos == len(s_c)
    return T, tile_off, int(ntiles), gidx, dstloc


def _wrap_idx(flat16):
    n = flat16.shape[0]
    w = flat16.reshape(n // 16, 16).T
    return np.ascontiguousarray(np.tile(w, (8, 1)), np.int16)


def _build_program(T, tile_off, ntiles):
    nc = bacc.Bacc(get_trn_type() or "TRN2", target_bir_lowering=False,
                   debug=False, enable_asserts=False, num_devices=NCORES,
                   num_swdge_queues=4)
    x_sh = nc.dram_tensor("x_shard", [NS, FEAT], F32, kind="ExternalInput").ap()
    gidx_t = nc.dram_tensor("gidx", [P, ntiles * 8], I16, kind="ExternalInput").ap()
    dstloc_t = nc.dram_tensor("dstloc", [P, ntiles], F32, kind="ExternalInput").ap()
    wext_t = [nc.dram_tensor(f"wext{l}", [FEAT, NCOL], F32, kind="ExternalInput").ap() for l in range(3)]
    btile_t = [nc.dram_tensor(f"btile{l}", [P, FEAT if l < 2 else OUT], F32, kind="ExternalInput").ap() for l in range(3)]
    iota_t = nc.dram_tensor("iota", [P, P], F32, kind="ExternalInput").ap()
    ident_t = nc.dram_tensor("ident", [P, P], F32, kind="ExternalInput").ap()
    out_t = nc.dram_tensor("out", [NS, OUT], F32, kind="ExternalOutput").ap()
    DBG = bool(int(os.environ.get("GAT_DEBUG", "0")))
    if DBG:
        dbg_xw = nc.dram_tensor("dbg_xw", [NS, NCOL], F32, kind="ExternalOutput").ap()
        dbg_h = nc.dram_tensor("dbg_h", [NS, FEAT], F32, kind="ExternalOutput").ap()
        dbg_den = nc.dram_tensor("dbg_den", [NS, 4], F32, kind="ExternalOutput").ap()
        dbg_ex = nc.dram_tensor("dbg_ex", [ntiles, P, 4], F32, kind="ExternalOutput").ap()
        dbg_sd = nc.dram_tensor("dbg_sd", [ntiles, P, 4], F32, kind="ExternalOutput").ap()

    groups = [list(range(NCORES))]
    NSUP = (NCH + S_CH - 1) // S_CH

    with tile.TileContext(nc) as tc:
        with (
            tc.tile_pool(name="const", bufs=1) as constp,
            tc.tile_pool(name="sched", bufs=1) as schedp,
            tc.tile_pool(name="gpool", bufs=6) as gpool,
            tc.tile_pool(name="eqp", bufs=3) as eqp,
            tc.tile_pool(name="eqtp", bufs=4) as eqtp,
            tc.tile_pool(name="work", bufs=4) as work,
            tc.tile_pool(name="nodep", bufs=3) as nodep,
            tc.tile_pool(name="accp", bufs=4, space="PSUM") as accp,
            tc.tile_pool(name="tpp", bufs=2, space="PSUM") as tpp,
            tc.tile_pool(name="sdp", bufs=2, space="PSUM") as sdp,
            tc.tile_pool(name="dram", bufs=2, space="DRAM") as dramp,
        ):
            iota = constp.tile([P, P], F32, tag="iota")
            nc.sync.dma_start(out=iota[:], in_=iota_t[:])
            ident = constp.tile([P, P], F32, tag="ident")
            nc.sync.dma_start(out=ident[:], in_=ident_t[:])
            wext, btile = [], []
            for l in range(3):
                w = constp.tile([FEAT, NCOL], F32, tag=f"wext{l}")
                nc.sync.dma_start(out=w[:], in_=wext_t[l][:])
                wext.append(w)
                b = constp.tile([P, FEAT if l < 2 else OUT], F32, tag=f"bt{l}")
                nc.sync.dma_start(out=b[:], in_=btile_t[l][:])
                btile.append(b)
            gidx_sb = schedp.tile([P, ntiles * 8], I16, tag="gidx")
            nc.sync.dma_start(out=gidx_sb[:], in_=gidx_t[:])
            dstloc_sb = schedp.tile([P, ntiles], F32, tag="dstloc")
            nc.sync.dma_start(out=dstloc_sb[:], in_=dstloc_t[:])

            xwss_sh = [dramp.tile([NS, ROWF], F32, tag="xwsh", name=f"xwsh{i}") for i in range(3)]
            xwss_full = [dramp.tile([N, ROWF], F32, tag="xwfull", name=f"xwfull{i}", addr_space="Shared") for i in range(3)]
            sd_sh = [dramp.tile([NS, 4], F32, tag="sdsh", name=f"sdsh{i}") for i in range(3)]

            def dense_tile(h_sb, lnext, base, nn):
                hT_ps = tpp.tile([P, P], F32, tag="tp")
                nc.tensor.transpose(out=hT_ps[:], in_=h_sb[:], identity=ident[:])
                hT_sb = work.tile([P, P], F32, tag="hT")
                nc.scalar.copy(out=hT_sb[:], in_=hT_ps[:])
                d_ps = tpp.tile([P, NCOL], F32, tag="tp", name="d_ps")
                nc.tensor.matmul(out=d_ps[:], lhsT=hT_sb[:], rhs=wext[lnext][:],
                                 start=True, stop=True)
                xo = work.tile([P, NCOL], F32, tag="xo")
                nc.scalar.copy(out=xo[:], in_=d_ps[:])
                nc.sync.dma_start(out=xwss_sh[lnext][base:base + nn, 0:NCOL],
                                  in_=xo[:nn, :])
                nc.sync.dma_start(out=sd_sh[lnext][base:base + nn, :],
                                  in_=xo[:nn, 132:136])
                if DBG and lnext == 0:
                    nc.sync.dma_start(out=dbg_xw[base:base + nn, :], in_=xo[:nn, :])

            for t in range(NCH):
                base = t * P
                nn = min(P, NS - base)
                xt = work.tile([P, FEAT], F32, tag="xt")
                nc.sync.dma_start(out=xt[:nn, :], in_=x_sh[base:base + nn, :])
                dense_tile(xt, 0, base, nn)

            for layer in range(3):
                nc.gpsimd.collective_compute(
                    "AllGather", mybir.AluOpType.bypass, replica_groups=groups,
                    ins=[xwss_sh[layer].opt()], outs=[xwss_full[layer].opt()])

                for sup in range(NSUP):
                    ch0 = sup * S_CH
                    chn = min(S_CH, NCH - ch0)

                    sdch = []
                    for ci in range(chn):
                        base = (ch0 + ci) * P
                        nn = min(P, NS - base)
                        s = work.tile([P, 4], F32, tag="sd")
                        nc.sync.dma_start(out=s[:nn, :], in_=sd_sh[layer][base:base + nn, :])
                        sdch.append(s)

                    # per src-chunk: contiguous tile span + one dma_gather
                    spans = []
                    for sc in range(NSC):
                        t0 = int(tile_off[ch0, sc])
                        ncall = int(sum(T[ch0 + ci, sc] for ci in range(chn)))
                        spans.append((t0, ncall))

                    acc_ps = [accp.tile([P, NCOL], F32, tag="acc", name=f"acc_l{layer}s{sup}c{ci}") for ci in range(chn)]
                    mm_count = [0] * chn
                    mm_total = [int(T[ch0 + ci, :].sum()) for ci in range(chn)]

                    for sc in range(NSC):
                        t0, ncall = spans[sc]
                        if ncall == 0:
                            continue
                        gout = gpool.tile([P, ncall, ROWF], F32, tag="g")
                        nc.gpsimd.dma_gather(
                            out_ap=gout[:],
                            in_ap=xwss_full[layer][sc * CH:(sc + 1) * CH, :],
                            idxs_ap=gidx_sb[:, t0 * 8:(t0 + ncall) * 8],
                            num_idxs=ncall * P, num_idxs_reg=ncall * P,
                            elem_size=ROWF, single_packet=False, queue_num=sc % 4)

                        eqg = eqp.tile([P, ncall, P], F32, tag="eq")
                        sd_ps = sdp.tile([P, ncall, 4], F32, tag="sdps")
                        slot = 0
                        tlist = []
                        for ci in range(chn):
                            for _ in range(int(T[ch0 + ci, sc])):
                                tg = t0 + slot
                                nc.vector.tensor_scalar(
                                    out=eqg[:, slot, :], in0=iota[:],
                                    scalar1=dstloc_sb[:, tg:tg + 1], scalar2=None,
                                    op0=mybir.AluOpType.is_equal)
                                eqT_ps = tpp.tile([P, P], F32, tag="tp")
                                nc.tensor.transpose(out=eqT_ps[:], in_=eqg[:, slot, :],
                                                    identity=ident[:])
                                eqT = eqtp.tile([P, P], F32, tag="eqT")
                                nc.scalar.copy(out=eqT[:], in_=eqT_ps[:])
                                nc.tensor.matmul(out=sd_ps[:, slot, :], lhsT=eqT[:],
                                                 rhs=sdch[ci][:], start=True, stop=True)
                                tlist.append(ci)
                                slot += 1

                        al = work.tile([P, ncall, 4], F32, tag="al")
                        nc.vector.tensor_tensor(out=al[:], in0=gout[:, :, 128:132],
                                                in1=sd_ps[:], op=mybir.AluOpType.add)
                        al2 = work.tile([P, ncall, 4], F32, tag="al2")
                        nc.vector.tensor_scalar(out=al2[:], in0=al[:], scalar1=SLOPE,
                                                scalar2=None, op0=mybir.AluOpType.mult)
                        nc.vector.tensor_tensor(out=al[:], in0=al[:], in1=al2[:],
                                                op=mybir.AluOpType.max)
                        nc.scalar.activation(out=gout[:, :, 132:136], in_=al[:],
                                             func=mybir.ActivationFunctionType.Exp)
                        nc.vector.tensor_tensor(
                            out=gout[:, :, 0:128].rearrange("p t (h c) -> p t h c", h=4),
                            in0=gout[:, :, 0:128].rearrange("p t (h c) -> p t h c", h=4),
                            in1=gout[:, :, 132:136].unsqueeze(3).broadcast_to([P, ncall, 4, 32]),
                            op=mybir.AluOpType.mult)

                        if DBG and layer == 0:
                            sdc = work.tile([P, ncall, 4], F32, tag="sdc")
                            nc.vector.tensor_copy(out=sdc[:], in_=sd_ps[:])
                            nc.sync.dma_start(out=dbg_sd[t0:t0 + ncall].rearrange("t p f -> p t f"), in_=sdc[:])
                            nc.sync.dma_start(out=dbg_ex[t0:t0 + ncall].rearrange("t p f -> p t f"), in_=gout[:, :, 132:136])
                        for slot, ci in enumerate(tlist):
                            mm_count[ci] += 1
                            nc.tensor.matmul(
                                out=acc_ps[ci][:], lhsT=eqg[:, slot, :],
                                rhs=gout[:, slot, 0:NCOL],
                                start=(mm_count[ci] == 1),
                                stop=(mm_count[ci] == mm_total[ci]))

                    for ci in range(chn):
                        base = (ch0 + ci) * P
                        nn = min(P, NS - base)
                        r = work.tile([P, 4], F32, tag="r")
                        if DBG and layer == 0:
                            dnt = work.tile([P, 4], F32, tag="dnt")
                            nc.vector.tensor_copy(out=dnt[:], in_=acc_ps[ci][:, 132:136])
                            nc.sync.dma_start(out=dbg_den[base:base + nn, :], in_=dnt[:nn, :])
                        nc.vector.reciprocal(out=r[:], in_=acc_ps[ci][:, 132:136])
                        h = nodep.tile([P, FEAT], F32, tag="h")
                        nc.vector.tensor_tensor(
                            out=h[:].rearrange("p (h c) -> p h c", h=4),
                            in0=acc_ps[ci][:, 0:128].rearrange("p (h c) -> p h c", h=4),
                            in1=r[:].unsqueeze(2).broadcast_to([P, 4, 32]),
                            op=mybir.AluOpType.mult)
                        if layer < 2:
                            nc.vector.tensor_tensor(out=h[:], in0=h[:], in1=btile[layer][:],
                                                    op=mybir.AluOpType.add)
                            mn = nodep.tile([P, FEAT], F32, tag="mn")
                            nc.vector.tensor_scalar(out=mn[:], in0=h[:], scalar1=0.0,
                                                    scalar2=None, op0=mybir.AluOpType.min)
                            nc.scalar.activation(out=mn[:], in_=mn[:],
                                                 func=mybir.ActivationFunctionType.Exp)
                            nc.vector.tensor_scalar(out=h[:], in0=h[:], scalar1=0.0,
                                                    scalar2=None, op0=mybir.AluOpType.max)
                            nc.vector.tensor_tensor(out=h[:], in0=h[:], in1=mn[:],
                                                    op=mybir.AluOpType.add)
                            nc.vector.tensor_scalar(out=h[:], in0=h[:], scalar1=-1.0,
                                                    scalar2=None, op0=mybir.AluOpType.add)
                            if DBG and layer == 0:
                                nc.sync.dma_start(out=dbg_h[base:base + nn, :], in_=h[:nn, :])
                            dense_tile(h, layer + 1, base, nn)
                        else:
                            o = nodep.tile([P, OUT], F32, tag="o")
                            hv = h[:].rearrange("p (h c) -> p h c", h=4)
                            nc.vector.tensor_tensor(out=o[:], in0=hv[:, 0, :], in1=hv[:, 1, :],
                                                    op=mybir.AluOpType.add)
                            nc.vector.tensor_tensor(out=o[:], in0=o[:], in1=hv[:, 2, :],
                                                    op=mybir.AluOpType.add)
                            nc.vector.tensor_tensor(out=o[:], in0=o[:], in1=hv[:, 3, :],
                                                    op=mybir.AluOpType.add)
                            nc.vector.tensor_scalar(out=o[:], in0=o[:], scalar1=0.25,
                                                    scalar2=None, op0=mybir.AluOpType.mult)
                            nc.vector.tensor_tensor(out=o[:], in0=o[:], in1=btile[2][:],
                                                    op=mybir.AluOpType.add)
                            nc.sync.dma_start(out=out_t[base:base + nn, :], in_=o[:nn, :])
    nc.compile()
    return nc


def kernel(x, edge_index, W1, as1, ad1, b1, W2, as2, ad2, b2, W3, as3, ad3, b3):
    x = np.asarray(x, np.float32)
    edge_index = np.asarray(edge_index)
    T, tile_off, ntiles, gidx, dstloc = _prep_graph(edge_index)
    nc = _build_program(T, tile_off, ntiles)

    wext = [_build_wext(np.asarray(W1, np.float32), np.asarray(as1, np.float32), np.asarray(ad1, np.float32)),
            _build_wext(np.asarray(W2, np.float32), np.asarray(as2, np.float32), np.asarray(ad2, np.float32)),
            _build_wext(np.asarray(W3, np.float32), np.asarray(as3, np.float32), np.asarray(ad3, np.float32))]
    bt = [np.ascontiguousarray(np.tile(np.asarray(b, np.float32)[None, :], (P, 1)))
          for b in (b1, b2, b3)]
    iota_np = np.ascontiguousarray(np.tile(np.arange(P, dtype=np.float32)[None, :], (P, 1)))
    ident_np = np.eye(P, dtype=np.float32)

    in_maps = []
    for c in range(NCORES):
        m = {
            "x_shard": np.ascontiguousarray(x[c * NS:(c + 1) * NS]),
            "gidx": _wrap_idx(gidx[c]),
            "dstloc": np.ascontiguousarray(dstloc[c].T),
            "iota": iota_np, "ident": ident_np,
        }
        for l in range(3):
            m[f"wext{l}"] = wext[l]
            m[f"btile{l}"] = bt[l]
        in_maps.append(m)

    trace = bool(int(os.environ.get("GAT_TRACE", "0")))
    res = run_bass_kernel_spmd(nc, in_maps, list(range(NCORES)), trace=trace)
    kernel.last_exec_time_ns = res.exec_time_ns
    out = np.concatenate([res.results[c]["out"] for c in range(NCORES)], axis=0)
    return out


kernel.last_exec_time_ns = None
